# revision 5
# baseline (speedup 1.0000x reference)
"""HashGrid embedding_lookup kernel for 8 trn2 NeuronCores — device-side hashing.

Data-parallel over 262144 points (32768/core, 8 Q7 groups x 4096, 16 chunks of
256 points). Uploads per core: coords in two layouts (0.8MB) + table fp16 (2MB).
On device per (chunk, level): hash corner indices on DVE (fp32 exact-int
products of the low-16-bit hash factors, int16 XOR tree, pre-shifted pair
indices), ap_gather fp16 feature pairs on GPSIMD, parity-blend + 3-stage
trilinear lerp on DVE (per-point weight streams broadcast to the 128
partitions via a group-selector matmul on PE), transpose feature tiles to
point-major on PE, and DMA fp16 features [32768, 256] per core. Host adds the
39-col positional encoding and casts to fp32."""

import numpy as np

L = 16
T = 65536
F = 16
COARSE = 16
FINE = 512
NUM_FREQ = 6
NCORES = 8
GROUPS = 8
PTS_NC = 32768
PTS_G = PTS_NC // GROUPS        # 4096
CHUNKS = 16
PTS_CH = PTS_G // CHUNKS        # 256
NIDX = PTS_CH * 8               # 2048 gather idx per Q7 core per (chunk, level)
QMAX = 4.0                      # int8 feature quantization range
QSCALE = 127.0 / QMAX

_b = np.float32(2.0) ** (np.log2(np.float32(FINE) / np.float32(COARSE)) / np.float32(L - 1))
NL = np.floor(np.float32(COARSE) * _b ** np.arange(L, dtype=np.float32)).astype(np.float32)
FACTORS = np.array([1, 2654435761, 805459861], dtype=np.uint64)
FLOW = [float(int(f) & 0xFFFF) for f in FACTORS]   # [1.0, 31153.0, 32405.0]

_COMPILED = {}


def _patch_drain():
    import concourse.mybir as mybir
    from concourse import tile

    def _patched_drain_and_barrier(self, tick_clock, wait_clock):
        drain_inst = self.nc.sync.drain()
        wait_clock.add_sem_waits(drain_inst.ins, tile.ScopedClock({None: tick_clock.global_clock}))
        si = drain_inst.ins.sync_info
        waits = list(si.on_wait or [])
        si.on_wait.clear()
        for w in waits:
            nop = self.nc.sync.nop(hint="drain_waits", nofuse=True)
            nsi = nop.ins.sync_info
            if nsi is None:
                nop.ins.sync_info = mybir.SyncInfo(on_wait=[w], on_update=[])
            else:
                nsi.on_wait.append(w)
        self.nc.all_engine_barrier()
        popped = self.nc._tile_sem_poison_stack.pop()
        assert popped is self._sem_poison
        self.nc.clear_and_free_semaphores(list(self.sems.allocated().values()))
        self.nc.all_engine_barrier()
    tile.TileContext._drain_and_barrier = _patched_drain_and_barrier


def _build_program(chunks=CHUNKS):
    import concourse.bacc as bacc
    import concourse.mybir as mybir
    from concourse import tile
    from concourse.masks import make_identity

    _patch_drain()
    A = mybir.AluOpType

    nc = bacc.Bacc()
    npts = 8 * chunks * PTS_CH
    tbl_h = nc.declare_dram_parameter("tbl", [16, T], mybir.dt.float16, isOutput=False)
    # [128=(g,p16), 3 axes, chunks, 16 p_hi]
    xtw_h = nc.declare_dram_parameter("xtw", [128, 3 * chunks * 16], mybir.dt.float32, isOutput=False)
    # [8=g, chunks * (3 axes * 256 p_ord)]
    xtf_h = nc.declare_dram_parameter("xtf", [8, chunks * 3 * PTS_CH], mybir.dt.float32, isOutput=False)
    sel_h = nc.declare_dram_parameter("sel", [8, 128], mybir.dt.float32, isOutput=False)
    scr_h = nc.declare_dram_parameter("scr", [npts, L * F], mybir.dt.int8, isOutput=True)

    with tile.TileContext(nc) as tc:
        with (
            tc.tile_pool(name="cst", bufs=1) as cst,
            tc.tile_pool(name="p1", bufs=1) as p1,
            tc.tile_pool(name="p2", bufs=2) as p2,
            tc.tile_pool(name="psA", bufs=2, space="PSUM") as psA,
            tc.tile_pool(name="psT", bufs=2, space="PSUM") as psT,
        ):
            # --- constants / resident data ---
            t_tbl = cst.tile([128, T], mybir.dt.float16, tag="tbl")
            for g in range(8):
                nc.sync.dma_start(out=t_tbl[16 * g:16 * (g + 1), :], in_=tbl_h[:, :])
            tbl_v = t_tbl.rearrange("p (e j) -> p e j", j=2)

            t_xtw = cst.tile([128, 3, chunks, 16], mybir.dt.float32, tag="xtw")
            nc.sync.dma_start(out=t_xtw.rearrange("p a c q -> p (a c q)"), in_=xtw_h[:, :])

            t_id = cst.tile([128, 128], mybir.dt.float16, tag="ident")
            make_identity(nc, t_id[:])

            t_sel = cst.tile([8, 128], mybir.dt.float32, tag="sel")
            nc.sync.dma_start(out=t_sel[:], in_=sel_h[:, :])

            t_nl = cst.tile([128, L], mybir.dt.float32, tag="nl")
            for l in range(L):
                nc.vector.memset(t_nl[:, l:l + 1], float(NL[l]))

            for cc in range(chunks):
                # --- per-point streams for this chunk, broadcast per group ---
                t_x8 = p1.tile([8, 3, PTS_CH], mybir.dt.float32, tag="x8")
                nc.sync.dma_start(
                    out=t_x8.rearrange("p a q -> p (a q)"),
                    in_=xtf_h[:, cc * 3 * PTS_CH:(cc + 1) * 3 * PTS_CH])
                p_xtf = psA.tile([128, 3, 512], mybir.dt.float32, tag="pxtf")
                for a in range(3):
                    nc.tensor.matmul(p_xtf[:, a, 0:PTS_CH], t_sel[:], t_x8[:, a, :],
                                     start=True, stop=True)

                # --- hash pipeline, two passes of 8 levels each ---
                # layout [128=(g,p16), 3, 8 l, 16 p_hi]
                xts = t_xtw[:, :, cc, :]                       # [128, 3, 16]
                L2 = L // 2
                t_iccs = []
                for lg in range(2):
                    lsl = slice(lg * L2, (lg + 1) * L2)
                    t_icc = p2.tile([128, L2, 8, 16], mybir.dt.int16, tag="icc")
                    t_iccs.append(t_icc)
                    t_sc = p1.tile([128, 3, L2, 16], mybir.dt.float32, tag="hsc")
                    nc.vector.tensor_tensor(
                        t_sc[:],
                        xts[:, :, None, :].to_broadcast([128, 3, L2, 16]),
                        t_nl[:, None, lsl, None].to_broadcast([128, 3, L2, 16]),
                        A.mult)
                    t_ri = p1.tile([128, 3, L2, 16], mybir.dt.int32, tag="hri")
                    nc.vector.tensor_copy(t_ri[:], t_sc[:])
                    t_rf = p1.tile([128, 3, L2, 16], mybir.dt.float32, tag="hrf")
                    nc.vector.tensor_copy(t_rf[:], t_ri[:])
                    t_g2 = p1.tile([128, 3, L2, 16], mybir.dt.float32, tag="hg2")
                    nc.vector.tensor_tensor(t_g2[:], t_rf[:], t_sc[:], A.is_gt)
                    t_lo = p1.tile([128, 3, L2, 16], mybir.dt.float32, tag="hlo")
                    nc.vector.tensor_tensor(t_lo[:], t_rf[:], t_g2[:], A.subtract)
                    t_gu = p1.tile([128, 3, L2, 16], mybir.dt.float32, tag="hgu")
                    nc.vector.tensor_tensor(t_gu[:], t_sc[:], t_lo[:], A.is_gt)
                    t_up = p1.tile([128, 3, L2, 16], mybir.dt.float32, tag="hup")
                    nc.vector.tensor_tensor(t_up[:], t_lo[:], t_gu[:], A.add)
                    # products (exact ints < 2^24), converted to int32 on output
                    t_pl = p1.tile([128, 3, L2, 16], mybir.dt.int32, tag="hpl")
                    t_ph = p1.tile([128, 3, L2, 16], mybir.dt.int32, tag="hph")
                    for a in range(3):
                        nc.vector.tensor_scalar(out=t_pl[:, a], in0=t_lo[:, a],
                                                scalar1=FLOW[a], scalar2=None, op0=A.mult)
                        nc.vector.tensor_scalar(out=t_ph[:, a], in0=t_up[:, a],
                                                scalar1=FLOW[a], scalar2=None, op0=A.mult)
                    # pre-shifted pair products: (p >> 1) & 0x7fff, then to int16
                    nc.vector.tensor_scalar(out=t_pl[:], in0=t_pl[:], scalar1=1,
                                            scalar2=0x7FFF, op0=A.logical_shift_right,
                                            op1=A.bitwise_and)
                    nc.vector.tensor_scalar(out=t_ph[:], in0=t_ph[:], scalar1=1,
                                            scalar2=0x7FFF, op0=A.logical_shift_right,
                                            op1=A.bitwise_and)
                    t_pl6 = p1.tile([128, 3, L2, 16], mybir.dt.int16, tag="hpl6")
                    t_ph6 = p1.tile([128, 3, L2, 16], mybir.dt.int16, tag="hph6")
                    nc.vector.tensor_copy(t_pl6[:], t_pl[:])
                    nc.vector.tensor_copy(t_ph6[:], t_ph[:])
                    # xy combos: [00, 11, 01, 10]
                    t_xy = p1.tile([128, 4, L2, 16], mybir.dt.int16, tag="hxy")
                    nc.vector.tensor_tensor(t_xy[:, 0], t_pl6[:, 0], t_pl6[:, 1], A.bitwise_xor)
                    nc.vector.tensor_tensor(t_xy[:, 1], t_ph6[:, 0], t_ph6[:, 1], A.bitwise_xor)
                    nc.vector.tensor_tensor(t_xy[:, 2], t_pl6[:, 0], t_ph6[:, 1], A.bitwise_xor)
                    nc.vector.tensor_tensor(t_xy[:, 3], t_ph6[:, 0], t_pl6[:, 1], A.bitwise_xor)
                    # slots (bx,by,bz): [000,110,011,101 | 001,111,010,100]
                    iv = t_icc[:].rearrange("p l s q -> p s l q")
                    zlo = t_pl6[:, 2, None].to_broadcast([128, 2, L2, 16])
                    zhi = t_ph6[:, 2, None].to_broadcast([128, 2, L2, 16])
                    nc.vector.tensor_tensor(iv[:, 0:2], t_xy[:, 0:2], zlo, A.bitwise_xor)
                    nc.vector.tensor_tensor(iv[:, 2:4], t_xy[:, 2:4], zhi, A.bitwise_xor)
                    nc.vector.tensor_tensor(iv[:, 4:6], t_xy[:, 0:2], zhi, A.bitwise_xor)
                    nc.vector.tensor_tensor(iv[:, 6:8], t_xy[:, 2:4], zlo, A.bitwise_xor)

                t_st = p1.tile([128, 2, 8, L, F], mybir.dt.int8, tag="st")

                for l in range(L):
                    # --- per-level weight streams (broadcast layout) ---
                    t_scb = p1.tile([128, 3, PTS_CH], mybir.dt.float32, tag="wscb")
                    nc.vector.tensor_scalar(out=t_scb[:], in0=p_xtf[:, :, 0:PTS_CH],
                                            scalar1=float(NL[l]), scalar2=None, op0=A.mult)
                    t_wri = p1.tile([128, 3, PTS_CH], mybir.dt.int32, tag="wri")
                    nc.vector.tensor_copy(t_wri[:], t_scb[:])
                    t_wrf = p1.tile([128, 3, PTS_CH], mybir.dt.float32, tag="wrf")
                    nc.vector.tensor_copy(t_wrf[:], t_wri[:])
                    t_wg2 = p1.tile([128, 3, PTS_CH], mybir.dt.float32, tag="wg2")
                    nc.vector.tensor_tensor(t_wg2[:], t_wrf[:], t_scb[:], A.is_gt)
                    # w = sc - (rf - g2) ;  li = int(rf - g2)
                    t_wt = p1.tile([128, 3, PTS_CH], mybir.dt.float32, tag="wt")
                    nc.vector.tensor_tensor(t_wt[:], t_scb[:], t_wrf[:], A.subtract)
                    t_wb = p1.tile([128, 3, PTS_CH], mybir.dt.float32, tag="wb")
                    nc.vector.tensor_tensor(t_wb[:], t_wt[:], t_wg2[:], A.add)
                    t_li = p1.tile([128, 3, PTS_CH], mybir.dt.int32, tag="wri")
                    nc.vector.tensor_tensor(t_li[:], t_wrf[:], t_wg2[:], A.subtract)
                    # basepar = (lx ^ ly ^ lz) & 1 -> fp16
                    t_bp32 = p1.tile([128, PTS_CH], mybir.dt.int32, tag="wt")
                    nc.vector.tensor_tensor(t_bp32[:], t_li[:, 0], t_li[:, 1], A.bitwise_xor)
                    nc.vector.tensor_tensor(t_bp32[:], t_bp32[:], t_li[:, 2], A.bitwise_xor)
                    nc.vector.tensor_scalar(out=t_bp32[:], in0=t_bp32[:], scalar1=1,
                                            scalar2=None, op0=A.bitwise_and)
                    t_bp = p1.tile([128, PTS_CH], mybir.dt.float16, tag="wri")
                    nc.vector.tensor_copy(t_bp[:], t_bp32[:])
                    # wz4 = [wz, wz, 1-wz, 1-wz]; wy2 = [wy, 1-wy]; wx
                    t_wz4 = p1.tile([128, 4, PTS_CH], mybir.dt.float16, tag="wg2")
                    nc.scalar.activation(t_wz4[:, 0:2], t_wb[:, 2, None, :].to_broadcast([128, 2, PTS_CH]),
                                         mybir.ActivationFunctionType.Copy)
                    nc.scalar.activation(t_wz4[:, 2:4], t_wb[:, 2, None, :].to_broadcast([128, 2, PTS_CH]),
                                         mybir.ActivationFunctionType.Copy, bias=1.0, scale=-1.0)
                    t_wy2 = p1.tile([128, 2, PTS_CH], mybir.dt.float16, tag="wrf")
                    nc.scalar.activation(t_wy2[:, 0:1], t_wb[:, 1, None, :].to_broadcast([128, 1, PTS_CH]),
                                         mybir.ActivationFunctionType.Copy)
                    nc.scalar.activation(t_wy2[:, 1:2], t_wb[:, 1, None, :].to_broadcast([128, 1, PTS_CH]),
                                         mybir.ActivationFunctionType.Copy, bias=1.0, scale=-1.0)
                    t_wx = p1.tile([128, PTS_CH], mybir.dt.float16, tag="wscb")
                    nc.scalar.activation(t_wx[:], t_wb[:, 0], mybir.ActivationFunctionType.Copy)

                    # --- gather ---
                    t_gg = p2.tile([128, NIDX * 2], mybir.dt.float16, tag="gg")
                    gv = t_gg.rearrange("p (k j) -> p k j", j=2)
                    nc.gpsimd.ap_gather(
                        gv, tbl_v,
                        t_iccs[l // L2][:, l % L2].rearrange("p s q -> p (s q)"),
                        channels=128, num_elems=T // 2, d=2, num_idxs=NIDX)

                    # --- parity blend + trilinear ---
                    H = NIDX // 2                                # 1024
                    t_td = p1.tile([128, NIDX], mybir.dt.float16, tag="btd")
                    nc.vector.tensor_tensor(t_td[:], gv[:, :, 1], gv[:, :, 0], A.subtract)
                    t_t2 = p1.tile([128, NIDX], mybir.dt.float16, tag="bt2")
                    nc.vector.tensor_tensor(
                        t_t2.rearrange("p (s q) -> p s q", s=8),
                        t_td.rearrange("p (s q) -> p s q", s=8),
                        t_bp[:, None, :].to_broadcast([128, 8, PTS_CH]),
                        A.mult)
                    t_v = p1.tile([128, NIDX], mybir.dt.float16, tag="btd")
                    nc.vector.tensor_tensor(t_v[:, 0:H], gv[:, 0:H, 0], t_t2[:, 0:H], A.add)
                    nc.vector.tensor_tensor(t_v[:, H:], gv[:, H:, 1], t_t2[:, H:], A.subtract)
                    t_dz = p1.tile([128, H], mybir.dt.float16, tag="bt2")
                    nc.vector.tensor_tensor(t_dz[:], t_v[:, H:], t_v[:, 0:H], A.subtract)
                    nc.vector.tensor_tensor(
                        t_dz.rearrange("p (s q) -> p s q", s=4),
                        t_dz.rearrange("p (s q) -> p s q", s=4),
                        t_wz4[:], A.mult)
                    t_z = p1.tile([128, H], mybir.dt.float16, tag="bz")
                    nc.vector.tensor_tensor(t_z[:], t_v[:, 0:H], t_dz[:], A.add)
                    t_dy = p1.tile([128, H // 2], mybir.dt.float16, tag="btd")
                    nc.vector.tensor_tensor(t_dy[:], t_z[:, H // 2:], t_z[:, 0:H // 2], A.subtract)
                    nc.vector.tensor_tensor(
                        t_dy.rearrange("p (s q) -> p s q", s=2),
                        t_dy.rearrange("p (s q) -> p s q", s=2),
                        t_wy2[:], A.mult)
                    t_y = p1.tile([128, H // 2], mybir.dt.float16, tag="bt2")
                    nc.vector.tensor_tensor(t_y[:], t_z[:, 0:H // 2], t_dy[:], A.add)
                    t_dx = p1.tile([128, PTS_CH], mybir.dt.float16, tag="bz2")
                    nc.vector.tensor_tensor(t_dx[:], t_y[:, PTS_CH:], t_y[:, 0:PTS_CH], A.subtract)
                    nc.vector.tensor_tensor(t_dx[:], t_dx[:], t_wx[:], A.mult)
                    t_f = p1.tile([128, PTS_CH], mybir.dt.float16, tag="bf")
                    nc.vector.tensor_tensor(t_f[:], t_y[:, 0:PTS_CH], t_dx[:], A.add)

                    # --- transpose to point-major ---
                    for b in range(2):
                        p_tp = psT.tile([128, 128], mybir.dt.float16, tag="ptp")
                        nc.tensor.transpose(p_tp[:], t_f[:, 128 * b:128 * (b + 1)], t_id[:])
                        nc.scalar.activation(
                            t_st[:, b, :, l, :],
                            p_tp.rearrange("p (g f) -> p g f", g=8),
                            mybir.ActivationFunctionType.Copy, scale=QSCALE)

                # --- chunk points out ---
                scr_v = scr_h[:, :].rearrange("(g c b p) q -> c b p g q",
                                              g=8, c=chunks, b=2, p=128)
                for b in range(2):
                    nc.sync.dma_start(
                        out=scr_v[cc, b],
                        in_=t_st[:, b].rearrange("p g l f -> p g (l f)"))

    nc.compile()
    return nc


def _pos_enc(xt):
    scales = (np.pi * 2.0 ** np.arange(NUM_FREQ)).astype(np.float32)
    ang = xt[..., None, :] * scales[:, None]                    # (P, 6, 3)
    pe = np.concatenate([np.sin(ang), np.cos(ang)], -1)         # (P, 6, 6)
    return np.concatenate([xt, pe.reshape(xt.shape[0], -1)], -1).astype(np.float32)


def kernel(x, t, tables, mask):
    from concourse.bass_utils import run_bass_kernel_spmd

    x = np.asarray(x); t = np.asarray(t)
    tables = np.asarray(tables); mask = np.asarray(mask)
    N, H, W, _ = x.shape

    flag = (mask == 0).astype(np.int64)
    order = np.argsort(flag, kind="stable")
    keep = order[:2]
    drop = int(order[2])

    coords = x[..., keep]                                       # (N,H,W,2)
    t_rep = np.broadcast_to(t[:, None, None, :], (N, H, W, 1))
    xt = np.concatenate([coords, t_rep], axis=-1).astype(np.float32).reshape(-1, 3)

    tbl16 = np.ascontiguousarray(tables[drop].astype(np.float16).T)   # (16, T)

    # device layouts
    v = xt.reshape(NCORES, GROUPS, CHUNKS, 16, 16, 3)       # (c, g, cc, p_hi, p16, a)
    xtw = np.ascontiguousarray(v.transpose(0, 1, 4, 5, 2, 3))  # (c, g, p16, a, cc, p_hi)
    xtw = xtw.reshape(NCORES, 128, 3 * CHUNKS * 16)
    v2 = xt.reshape(NCORES, GROUPS, CHUNKS, PTS_CH, 3)      # (c, g, cc, p_ord, a)
    xtf = np.ascontiguousarray(v2.transpose(0, 1, 2, 4, 3))  # (c, g, cc, a, p_ord)
    xtf = xtf.reshape(NCORES, 8, CHUNKS * 3 * PTS_CH)

    CH2 = CHUNKS // 2
    key = "prog_half"
    if key not in _COMPILED:
        _COMPILED[key] = _build_program(CH2)
    nc = _COMPILED[key]

    sel = np.zeros((8, 128), np.float32)
    for g in range(8):
        sel[g, 16 * g:16 * (g + 1)] = 1.0

    # split chunk dim: xtw (c, 128, 3, CHUNKS, 16); xtf (c, 8, CHUNKS, 3*256)
    xtw4 = xtw.reshape(NCORES, 128, 3, CHUNKS, 16)
    xtf4 = xtf.reshape(NCORES, 8, CHUNKS, 3 * PTS_CH)
    in_maps_h = []
    for h in range(2):
        csl = slice(h * CH2, (h + 1) * CH2)
        xtw_h = np.ascontiguousarray(xtw4[:, :, :, csl, :]).reshape(NCORES, 128, 3 * CH2 * 16)
        xtf_h = np.ascontiguousarray(xtf4[:, :, csl, :]).reshape(NCORES, 8, CH2 * 3 * PTS_CH)
        in_maps_h.append([
            {"tbl": tbl16, "xtw": xtw_h[c], "xtf": xtf_h[c], "sel": sel}
            for c in range(NCORES)])

    out = np.empty((N * H * W, L * F + 39), np.float32)

    import threading
    from concurrent.futures import ThreadPoolExecutor

    def _enc_work():
        out[:, L * F:] = _pos_enc(xt)
    enc_thr = threading.Thread(target=_enc_work)
    enc_thr.start()

    dq = np.float32(QMAX / 127.0)
    PTS_H = 8 * CH2 * PTS_CH          # points per half per core

    def _run_half(h):
        res = run_bass_kernel_spmd(nc, in_maps_h[h], list(range(NCORES)))
        for c in range(NCORES):
            scr = res.results[c]["scr"]           # (PTS_H, 256) int8
            for g in range(GROUPS):
                r0 = c * PTS_NC + g * PTS_G + h * PTS_H // GROUPS
                np.multiply(scr[g * (PTS_H // GROUPS):(g + 1) * (PTS_H // GROUPS)],
                            dq, out=out[r0:r0 + PTS_H // GROUPS, :L * F])

    with ThreadPoolExecutor(max_workers=2) as ex:
        list(ex.map(_run_half, range(2)))
    enc_thr.join()
    return out.reshape(N, H, W, L * F + 39)


# revision 6
# speedup vs baseline: 2.4183x; 2.4183x over previous
"""HashGrid embedding_lookup kernel for 8 trn2 NeuronCores — device-side hashing.

Data-parallel over 262144 points (32768/core, 8 Q7 groups x 4096, 16 chunks of
256 points). Uploads per core: coords in two layouts (0.8MB) + table fp16 (2MB).
On device per (chunk, level): hash corner indices on DVE (fp32 exact-int
products of the low-16-bit hash factors, int16 XOR tree, pre-shifted pair
indices), ap_gather fp16 feature pairs on GPSIMD, parity-blend + 3-stage
trilinear lerp on DVE (per-point weight streams broadcast to the 128
partitions via a group-selector matmul on PE), transpose feature tiles to
point-major on PE, and DMA fp16 features [32768, 256] per core. Host adds the
39-col positional encoding and casts to fp32."""

import numpy as np

L = 16
T = 65536
F = 16
COARSE = 16
FINE = 512
NUM_FREQ = 6
NCORES = 8
GROUPS = 8
PTS_NC = 32768
PTS_G = PTS_NC // GROUPS        # 4096
CHUNKS = 16
PTS_CH = PTS_G // CHUNKS        # 256
NIDX = PTS_CH * 8               # 2048 gather idx per Q7 core per (chunk, level)
QMAX = 4.0                      # int8 feature quantization range
QSCALE = 127.0 / QMAX

_b = np.float32(2.0) ** (np.log2(np.float32(FINE) / np.float32(COARSE)) / np.float32(L - 1))
NL = np.floor(np.float32(COARSE) * _b ** np.arange(L, dtype=np.float32)).astype(np.float32)
FACTORS = np.array([1, 2654435761, 805459861], dtype=np.uint64)
FLOW = [float(int(f) & 0xFFFF) for f in FACTORS]   # [1.0, 31153.0, 32405.0]

_COMPILED = {}


def _patch_drain():
    import concourse.mybir as mybir
    from concourse import tile

    def _patched_drain_and_barrier(self, tick_clock, wait_clock):
        drain_inst = self.nc.sync.drain()
        wait_clock.add_sem_waits(drain_inst.ins, tile.ScopedClock({None: tick_clock.global_clock}))
        si = drain_inst.ins.sync_info
        waits = list(si.on_wait or [])
        si.on_wait.clear()
        for w in waits:
            nop = self.nc.sync.nop(hint="drain_waits", nofuse=True)
            nsi = nop.ins.sync_info
            if nsi is None:
                nop.ins.sync_info = mybir.SyncInfo(on_wait=[w], on_update=[])
            else:
                nsi.on_wait.append(w)
        self.nc.all_engine_barrier()
        popped = self.nc._tile_sem_poison_stack.pop()
        assert popped is self._sem_poison
        self.nc.clear_and_free_semaphores(list(self.sems.allocated().values()))
        self.nc.all_engine_barrier()
    tile.TileContext._drain_and_barrier = _patched_drain_and_barrier


def _build_program(chunks=CHUNKS):
    import concourse.bacc as bacc
    import concourse.mybir as mybir
    from concourse import tile
    from concourse.masks import make_identity

    _patch_drain()
    A = mybir.AluOpType

    nc = bacc.Bacc()
    npts = 8 * chunks * PTS_CH
    tbl_h = nc.declare_dram_parameter("tbl", [16, T], mybir.dt.float16, isOutput=False)
    # [128=(g,p16), 3 axes, chunks, 16 p_hi]
    xtw_h = nc.declare_dram_parameter("xtw", [128, 3 * chunks * 16], mybir.dt.float32, isOutput=False)
    # [8=g, chunks * (3 axes * 256 p_ord)]
    xtf_h = nc.declare_dram_parameter("xtf", [8, chunks * 3 * PTS_CH], mybir.dt.float32, isOutput=False)
    sel_h = nc.declare_dram_parameter("sel", [8, 128], mybir.dt.float32, isOutput=False)
    scr_h = nc.declare_dram_parameter("scr", [npts, L * F], mybir.dt.int8, isOutput=True)

    with tile.TileContext(nc) as tc:
        with (
            tc.tile_pool(name="cst", bufs=1) as cst,
            tc.tile_pool(name="p1", bufs=1) as p1,
            tc.tile_pool(name="p2", bufs=2) as p2,
            tc.tile_pool(name="psA", bufs=2, space="PSUM") as psA,
            tc.tile_pool(name="psT", bufs=2, space="PSUM") as psT,
        ):
            # --- constants / resident data ---
            t_tbl = cst.tile([128, T], mybir.dt.float16, tag="tbl")
            for g in range(8):
                nc.sync.dma_start(out=t_tbl[16 * g:16 * (g + 1), :], in_=tbl_h[:, :])
            tbl_v = t_tbl.rearrange("p (e j) -> p e j", j=2)

            t_xtw = cst.tile([128, 3, chunks, 16], mybir.dt.float32, tag="xtw")
            nc.sync.dma_start(out=t_xtw.rearrange("p a c q -> p (a c q)"), in_=xtw_h[:, :])

            t_id = cst.tile([128, 128], mybir.dt.float16, tag="ident")
            make_identity(nc, t_id[:])

            t_sel = cst.tile([8, 128], mybir.dt.float32, tag="sel")
            nc.sync.dma_start(out=t_sel[:], in_=sel_h[:, :])

            t_nl = cst.tile([128, L], mybir.dt.float32, tag="nl")
            for l in range(L):
                nc.vector.memset(t_nl[:, l:l + 1], float(NL[l]))

            for cc in range(chunks):
                # --- per-point streams for this chunk, broadcast per group ---
                t_x8 = p1.tile([8, 3, PTS_CH], mybir.dt.float32, tag="x8")
                nc.sync.dma_start(
                    out=t_x8.rearrange("p a q -> p (a q)"),
                    in_=xtf_h[:, cc * 3 * PTS_CH:(cc + 1) * 3 * PTS_CH])
                p_xtf = psA.tile([128, 3, 512], mybir.dt.float32, tag="pxtf")
                for a in range(3):
                    nc.tensor.matmul(p_xtf[:, a, 0:PTS_CH], t_sel[:], t_x8[:, a, :],
                                     start=True, stop=True)

                # --- hash pipeline, two passes of 8 levels each ---
                # layout [128=(g,p16), 3, 8 l, 16 p_hi]
                xts = t_xtw[:, :, cc, :]                       # [128, 3, 16]
                L2 = L // 2
                t_iccs = []
                for lg in range(2):
                    lsl = slice(lg * L2, (lg + 1) * L2)
                    t_icc = p2.tile([128, L2, 8, 16], mybir.dt.int16, tag="icc")
                    t_iccs.append(t_icc)
                    t_sc = p1.tile([128, 3, L2, 16], mybir.dt.float32, tag="hsc")
                    nc.vector.tensor_tensor(
                        t_sc[:],
                        xts[:, :, None, :].to_broadcast([128, 3, L2, 16]),
                        t_nl[:, None, lsl, None].to_broadcast([128, 3, L2, 16]),
                        A.mult)
                    t_ri = p1.tile([128, 3, L2, 16], mybir.dt.int32, tag="hri")
                    nc.vector.tensor_copy(t_ri[:], t_sc[:])
                    t_rf = p1.tile([128, 3, L2, 16], mybir.dt.float32, tag="hrf")
                    nc.vector.tensor_copy(t_rf[:], t_ri[:])
                    t_g2 = p1.tile([128, 3, L2, 16], mybir.dt.float32, tag="hg2")
                    nc.vector.tensor_tensor(t_g2[:], t_rf[:], t_sc[:], A.is_gt)
                    t_lo = p1.tile([128, 3, L2, 16], mybir.dt.float32, tag="hlo")
                    nc.vector.tensor_tensor(t_lo[:], t_rf[:], t_g2[:], A.subtract)
                    t_gu = p1.tile([128, 3, L2, 16], mybir.dt.float32, tag="hgu")
                    nc.vector.tensor_tensor(t_gu[:], t_sc[:], t_lo[:], A.is_gt)
                    t_up = p1.tile([128, 3, L2, 16], mybir.dt.float32, tag="hup")
                    nc.vector.tensor_tensor(t_up[:], t_lo[:], t_gu[:], A.add)
                    # products (exact ints < 2^24), converted to int32 on output
                    t_pl = p1.tile([128, 3, L2, 16], mybir.dt.int32, tag="hpl")
                    t_ph = p1.tile([128, 3, L2, 16], mybir.dt.int32, tag="hph")
                    for a in range(3):
                        nc.vector.tensor_scalar(out=t_pl[:, a], in0=t_lo[:, a],
                                                scalar1=FLOW[a], scalar2=None, op0=A.mult)
                        nc.vector.tensor_scalar(out=t_ph[:, a], in0=t_up[:, a],
                                                scalar1=FLOW[a], scalar2=None, op0=A.mult)
                    # pre-shifted pair products: (p >> 1) & 0x7fff, then to int16
                    nc.vector.tensor_scalar(out=t_pl[:], in0=t_pl[:], scalar1=1,
                                            scalar2=0x7FFF, op0=A.logical_shift_right,
                                            op1=A.bitwise_and)
                    nc.vector.tensor_scalar(out=t_ph[:], in0=t_ph[:], scalar1=1,
                                            scalar2=0x7FFF, op0=A.logical_shift_right,
                                            op1=A.bitwise_and)
                    t_pl6 = p1.tile([128, 3, L2, 16], mybir.dt.int16, tag="hpl6")
                    t_ph6 = p1.tile([128, 3, L2, 16], mybir.dt.int16, tag="hph6")
                    nc.vector.tensor_copy(t_pl6[:], t_pl[:])
                    nc.vector.tensor_copy(t_ph6[:], t_ph[:])
                    # xy combos: [00, 11, 01, 10]
                    t_xy = p1.tile([128, 4, L2, 16], mybir.dt.int16, tag="hxy")
                    nc.vector.tensor_tensor(t_xy[:, 0], t_pl6[:, 0], t_pl6[:, 1], A.bitwise_xor)
                    nc.vector.tensor_tensor(t_xy[:, 1], t_ph6[:, 0], t_ph6[:, 1], A.bitwise_xor)
                    nc.vector.tensor_tensor(t_xy[:, 2], t_pl6[:, 0], t_ph6[:, 1], A.bitwise_xor)
                    nc.vector.tensor_tensor(t_xy[:, 3], t_ph6[:, 0], t_pl6[:, 1], A.bitwise_xor)
                    # slots (bx,by,bz): [000,110,011,101 | 001,111,010,100]
                    iv = t_icc[:].rearrange("p l s q -> p s l q")
                    zlo = t_pl6[:, 2, None].to_broadcast([128, 2, L2, 16])
                    zhi = t_ph6[:, 2, None].to_broadcast([128, 2, L2, 16])
                    nc.vector.tensor_tensor(iv[:, 0:2], t_xy[:, 0:2], zlo, A.bitwise_xor)
                    nc.vector.tensor_tensor(iv[:, 2:4], t_xy[:, 2:4], zhi, A.bitwise_xor)
                    nc.vector.tensor_tensor(iv[:, 4:6], t_xy[:, 0:2], zhi, A.bitwise_xor)
                    nc.vector.tensor_tensor(iv[:, 6:8], t_xy[:, 2:4], zlo, A.bitwise_xor)

                t_st = p1.tile([128, 2, 8, L, F], mybir.dt.int8, tag="st")

                for l in range(L):
                    # --- per-level weight streams (broadcast layout) ---
                    t_scb = p1.tile([128, 3, PTS_CH], mybir.dt.float32, tag="wscb")
                    nc.vector.tensor_scalar(out=t_scb[:], in0=p_xtf[:, :, 0:PTS_CH],
                                            scalar1=float(NL[l]), scalar2=None, op0=A.mult)
                    t_wri = p1.tile([128, 3, PTS_CH], mybir.dt.int32, tag="wri")
                    nc.vector.tensor_copy(t_wri[:], t_scb[:])
                    t_wrf = p1.tile([128, 3, PTS_CH], mybir.dt.float32, tag="wrf")
                    nc.vector.tensor_copy(t_wrf[:], t_wri[:])
                    t_wg2 = p1.tile([128, 3, PTS_CH], mybir.dt.float32, tag="wg2")
                    nc.vector.tensor_tensor(t_wg2[:], t_wrf[:], t_scb[:], A.is_gt)
                    # w = sc - (rf - g2) ;  li = int(rf - g2)
                    t_wt = p1.tile([128, 3, PTS_CH], mybir.dt.float32, tag="wt")
                    nc.vector.tensor_tensor(t_wt[:], t_scb[:], t_wrf[:], A.subtract)
                    t_wb = p1.tile([128, 3, PTS_CH], mybir.dt.float32, tag="wb")
                    nc.vector.tensor_tensor(t_wb[:], t_wt[:], t_wg2[:], A.add)
                    t_li = p1.tile([128, 3, PTS_CH], mybir.dt.int32, tag="wri")
                    nc.vector.tensor_tensor(t_li[:], t_wrf[:], t_wg2[:], A.subtract)
                    # basepar = (lx ^ ly ^ lz) & 1 -> fp16
                    t_bp32 = p1.tile([128, PTS_CH], mybir.dt.int32, tag="wt")
                    nc.vector.tensor_tensor(t_bp32[:], t_li[:, 0], t_li[:, 1], A.bitwise_xor)
                    nc.vector.tensor_tensor(t_bp32[:], t_bp32[:], t_li[:, 2], A.bitwise_xor)
                    nc.vector.tensor_scalar(out=t_bp32[:], in0=t_bp32[:], scalar1=1,
                                            scalar2=None, op0=A.bitwise_and)
                    t_bp = p1.tile([128, PTS_CH], mybir.dt.float16, tag="wri")
                    nc.vector.tensor_copy(t_bp[:], t_bp32[:])
                    # wz4 = [wz, wz, 1-wz, 1-wz]; wy2 = [wy, 1-wy]; wx
                    t_wz4 = p1.tile([128, 4, PTS_CH], mybir.dt.float16, tag="wg2")
                    nc.scalar.activation(t_wz4[:, 0:2], t_wb[:, 2, None, :].to_broadcast([128, 2, PTS_CH]),
                                         mybir.ActivationFunctionType.Copy)
                    nc.scalar.activation(t_wz4[:, 2:4], t_wb[:, 2, None, :].to_broadcast([128, 2, PTS_CH]),
                                         mybir.ActivationFunctionType.Copy, bias=1.0, scale=-1.0)
                    t_wy2 = p1.tile([128, 2, PTS_CH], mybir.dt.float16, tag="wrf")
                    nc.scalar.activation(t_wy2[:, 0:1], t_wb[:, 1, None, :].to_broadcast([128, 1, PTS_CH]),
                                         mybir.ActivationFunctionType.Copy)
                    nc.scalar.activation(t_wy2[:, 1:2], t_wb[:, 1, None, :].to_broadcast([128, 1, PTS_CH]),
                                         mybir.ActivationFunctionType.Copy, bias=1.0, scale=-1.0)
                    t_wx = p1.tile([128, PTS_CH], mybir.dt.float16, tag="wscb")
                    nc.scalar.activation(t_wx[:], t_wb[:, 0], mybir.ActivationFunctionType.Copy)

                    # --- gather ---
                    t_gg = p2.tile([128, NIDX * 2], mybir.dt.float16, tag="gg")
                    gv = t_gg.rearrange("p (k j) -> p k j", j=2)
                    nc.gpsimd.ap_gather(
                        gv, tbl_v,
                        t_iccs[l // L2][:, l % L2].rearrange("p s q -> p (s q)"),
                        channels=128, num_elems=T // 2, d=2, num_idxs=NIDX)

                    # --- parity blend + trilinear ---
                    H = NIDX // 2                                # 1024
                    t_td = p1.tile([128, NIDX], mybir.dt.float16, tag="btd")
                    nc.vector.tensor_tensor(t_td[:], gv[:, :, 1], gv[:, :, 0], A.subtract)
                    t_t2 = p1.tile([128, NIDX], mybir.dt.float16, tag="bt2")
                    nc.vector.tensor_tensor(
                        t_t2.rearrange("p (s q) -> p s q", s=8),
                        t_td.rearrange("p (s q) -> p s q", s=8),
                        t_bp[:, None, :].to_broadcast([128, 8, PTS_CH]),
                        A.mult)
                    t_v = p1.tile([128, NIDX], mybir.dt.float16, tag="btd")
                    nc.vector.tensor_tensor(t_v[:, 0:H], gv[:, 0:H, 0], t_t2[:, 0:H], A.add)
                    nc.vector.tensor_tensor(t_v[:, H:], gv[:, H:, 1], t_t2[:, H:], A.subtract)
                    t_dz = p1.tile([128, H], mybir.dt.float16, tag="bt2")
                    nc.vector.tensor_tensor(t_dz[:], t_v[:, H:], t_v[:, 0:H], A.subtract)
                    nc.vector.tensor_tensor(
                        t_dz.rearrange("p (s q) -> p s q", s=4),
                        t_dz.rearrange("p (s q) -> p s q", s=4),
                        t_wz4[:], A.mult)
                    t_z = p1.tile([128, H], mybir.dt.float16, tag="bz")
                    nc.vector.tensor_tensor(t_z[:], t_v[:, 0:H], t_dz[:], A.add)
                    t_dy = p1.tile([128, H // 2], mybir.dt.float16, tag="btd")
                    nc.vector.tensor_tensor(t_dy[:], t_z[:, H // 2:], t_z[:, 0:H // 2], A.subtract)
                    nc.vector.tensor_tensor(
                        t_dy.rearrange("p (s q) -> p s q", s=2),
                        t_dy.rearrange("p (s q) -> p s q", s=2),
                        t_wy2[:], A.mult)
                    t_y = p1.tile([128, H // 2], mybir.dt.float16, tag="bt2")
                    nc.vector.tensor_tensor(t_y[:], t_z[:, 0:H // 2], t_dy[:], A.add)
                    t_dx = p1.tile([128, PTS_CH], mybir.dt.float16, tag="bz2")
                    nc.vector.tensor_tensor(t_dx[:], t_y[:, PTS_CH:], t_y[:, 0:PTS_CH], A.subtract)
                    nc.vector.tensor_tensor(t_dx[:], t_dx[:], t_wx[:], A.mult)
                    t_f = p1.tile([128, PTS_CH], mybir.dt.float16, tag="bf")
                    nc.vector.tensor_tensor(t_f[:], t_y[:, 0:PTS_CH], t_dx[:], A.add)

                    # --- transpose to point-major ---
                    for b in range(2):
                        p_tp = psT.tile([128, 128], mybir.dt.float16, tag="ptp")
                        nc.tensor.transpose(p_tp[:], t_f[:, 128 * b:128 * (b + 1)], t_id[:])
                        nc.scalar.activation(
                            t_st[:, b, :, l, :],
                            p_tp.rearrange("p (g f) -> p g f", g=8),
                            mybir.ActivationFunctionType.Copy, scale=QSCALE)

                # --- chunk points out ---
                scr_v = scr_h[:, :].rearrange("(g c b p) q -> c b p g q",
                                              g=8, c=chunks, b=2, p=128)
                for b in range(2):
                    nc.sync.dma_start(
                        out=scr_v[cc, b],
                        in_=t_st[:, b].rearrange("p g l f -> p g (l f)"))

    nc.compile()
    return nc


def _pos_enc(xt):
    scales = (np.pi * 2.0 ** np.arange(NUM_FREQ)).astype(np.float32)
    ang = xt[..., None, :] * scales[:, None]                    # (P, 6, 3)
    pe = np.concatenate([np.sin(ang), np.cos(ang)], -1)         # (P, 6, 6)
    return np.concatenate([xt, pe.reshape(xt.shape[0], -1)], -1).astype(np.float32)


def kernel(x, t, tables, mask):
    from concourse.bass_utils import run_bass_kernel_spmd

    x = np.asarray(x); t = np.asarray(t)
    tables = np.asarray(tables); mask = np.asarray(mask)
    N, H, W, _ = x.shape

    flag = (mask == 0).astype(np.int64)
    order = np.argsort(flag, kind="stable")
    keep = order[:2]
    drop = int(order[2])

    coords = x[..., keep]                                       # (N,H,W,2)
    t_rep = np.broadcast_to(t[:, None, None, :], (N, H, W, 1))
    xt = np.concatenate([coords, t_rep], axis=-1).astype(np.float32).reshape(-1, 3)

    tbl16 = np.ascontiguousarray(tables[drop].astype(np.float16).T)   # (16, T)

    # device layouts
    v = xt.reshape(NCORES, GROUPS, CHUNKS, 16, 16, 3)       # (c, g, cc, p_hi, p16, a)
    xtw = np.ascontiguousarray(v.transpose(0, 1, 4, 5, 2, 3))  # (c, g, p16, a, cc, p_hi)
    xtw = xtw.reshape(NCORES, 128, 3 * CHUNKS * 16)
    v2 = xt.reshape(NCORES, GROUPS, CHUNKS, PTS_CH, 3)      # (c, g, cc, p_ord, a)
    xtf = np.ascontiguousarray(v2.transpose(0, 1, 2, 4, 3))  # (c, g, cc, a, p_ord)
    xtf = xtf.reshape(NCORES, 8, CHUNKS * 3 * PTS_CH)

    key = "prog"
    if key not in _COMPILED:
        _COMPILED[key] = _build_program()
    nc = _COMPILED[key]

    sel = np.zeros((8, 128), np.float32)
    for g in range(8):
        sel[g, 16 * g:16 * (g + 1)] = 1.0
    in_maps = [{"tbl": tbl16, "xtw": xtw[c], "xtf": xtf[c], "sel": sel}
               for c in range(NCORES)]

    out = np.empty((N * H * W, L * F + 39), np.float32)

    import threading
    from concurrent.futures import ThreadPoolExecutor

    def _enc_work():
        out[:, L * F:] = _pos_enc(xt)
    enc_thr = threading.Thread(target=_enc_work)
    enc_thr.start()

    res = run_bass_kernel_spmd(nc, in_maps, list(range(NCORES)))

    dq = np.float32(QMAX / 127.0)

    def _copy(c):
        np.multiply(res.results[c]["scr"], dq,
                    out=out[c * PTS_NC:(c + 1) * PTS_NC, :L * F])
    with ThreadPoolExecutor(max_workers=NCORES) as ex:
        list(ex.map(_copy, range(NCORES)))
    enc_thr.join()
    return out.reshape(N, H, W, L * F + 39)
